# revision 2
# baseline (speedup 1.0000x reference)
"""ArcFace loss TRN2 kernel: 8-core class-parallel (tensor-parallel over
num_classes), f32r matmul, on-device weight normalization and sum-exp.

kernel(embeddings, labels, weight) -> (loss, output)
  embeddings (512, 512) f32, labels (512,) int, weight (100000, 512) f32
  output (512, 100000) f32 = ArcFace-scaled logits, loss = scalar CE.

Per core c: classes [c*12500, (c+1)*12500), padded to 12544 on device.
Device computes out = (S * emb_n) @ w_n.T for its class shard plus
rowsum(exp(out)); host applies the one-hot phi fix-up (512 cells), merges
the per-core sum-exp, and computes the scalar loss.
"""
import sys

sys.path.insert(0, "/opt/trn_rl_repo")

import math
import numpy as np

B = 512          # batch
E = 512          # embedding dim
C = 100000       # num classes
NCORES = 8
CPC = C // NCORES            # 12500 classes per core
CPC_PAD = 12544              # 98 chunks of 128
CT_SIZES = [512] * 24 + [256]   # class free-tiles per core (24*512 + 256)
PAD_COLS = CPC_PAD - CPC     # 44 zero-padded class columns per core

S = 30.0
MARGIN = 0.5
COS_M = math.cos(MARGIN)
SIN_M = math.sin(MARGIN)
TH = math.cos(math.pi - MARGIN)
MM = math.sin(math.pi - MARGIN) * MARGIN

_CACHE = {}


def _build():
    import concourse.bass as bass
    import concourse.tile as tile
    from concourse import bacc, mybir
    from concourse.masks import make_identity

    f32 = mybir.dt.float32
    f32r = mybir.dt.float32r
    P = 128

    nc = bacc.Bacc(None)
    emb = nc.declare_dram_parameter("emb", [B, E], f32, isOutput=False)
    w = nc.declare_dram_parameter("w", [CPC, E], f32, isOutput=False)
    out = nc.declare_dram_parameter("out", [B, CPC_PAD], f32, isOutput=True)
    sumexp = nc.declare_dram_parameter("sumexp", [B], f32, isOutput=True)

    out_v = out.rearrange("(m p) c -> p m c", p=P)        # (128, 4, 12544)
    emb_v = emb.rearrange("(t p) e -> p t e", p=P)        # (128, 4, 512)

    with tile.TileContext(nc) as tc:
        with (
            tc.tile_pool(name="persist", bufs=1) as persist,
            tc.tile_pool(name="wstage", bufs=3) as wstage_pool,
            tc.tile_pool(name="wn", bufs=2) as wn_pool,
            tc.tile_pool(name="wt", bufs=2) as wt_pool,
            tc.tile_pool(name="ostage", bufs=3) as ostage_pool,
            tc.tile_pool(name="scratch", bufs=2) as scratch,
            tc.tile_pool(name="small", bufs=4) as small,
            tc.tile_pool(name="pst", bufs=2, space="PSUM") as pst_pool,
            tc.tile_pool(name="pso", bufs=6, space="PSUM") as pso_pool,
        ):
            ident_f = persist.tile([P, P], f32)
            make_identity(nc, ident_f)
            ident = persist.tile([P, P], f32r)
            nc.vector.tensor_copy(ident[:], ident_f[:])

            # ---- embeddings: load, l2-normalize rows, scale by S, transpose
            emb_sb = persist.tile([P, 4, E], f32)
            nc.sync.dma_start(emb_sb[:], emb_v[:])
            ess = persist.tile([P, 4], f32)
            sq_scr = scratch.tile([P, E], f32, tag="sq")
            for t in range(4):
                sq_scr = scratch.tile([P, E], f32, tag="sq")
                nc.vector.scalar_tensor_tensor(
                    out=sq_scr[:], in0=emb_sb[:, t, :], scalar=1.0,
                    in1=emb_sb[:, t, :],
                    op0=mybir.AluOpType.mult, op1=mybir.AluOpType.mult,
                    accum_out=ess[:, t:t + 1],
                )
            enorm = persist.tile([P, 4], f32)
            nc.scalar.activation(enorm[:], ess[:], mybir.ActivationFunctionType.Sqrt)
            nc.vector.tensor_scalar_max(enorm[:], enorm[:], 1e-12)
            ers = persist.tile([P, 4], f32)
            nc.vector.reciprocal(ers[:], enorm[:])
            embS = persist.tile([P, 4, E], f32r)
            for t in range(4):
                nc.vector.tensor_scalar(
                    out=embS[:, t, :], in0=emb_sb[:, t, :],
                    scalar1=ers[:, t:t + 1], scalar2=S,
                    op0=mybir.AluOpType.mult, op1=mybir.AluOpType.mult,
                )
            embT = persist.tile([P, 4, B], f32r)   # [e%128, e//128, b]
            for k in range(4):
                ps_e = pst_pool.tile([P, B], f32, tag="tpose")
                for t in range(4):
                    nc.tensor.matmul(
                        ps_e[:, t * P:(t + 1) * P].bitcast(f32r),
                        embS[:, t, k * P:(k + 1) * P],
                        ident[:],
                        is_transpose=True,
                    )
                nc.vector.tensor_copy(embT[:, k, :], ps_e[:].bitcast(f32r))

            # ---- expsum collector: [p, m*32 + ct]
            expsums = persist.tile([P, 4 * 32], f32)

            # ---- main loop over class tiles
            col = 0
            for ct, SZ in enumerate(CT_SIZES):
                NCH = SZ // P  # 128-class chunks in this tile
                w_stage = wstage_pool.tile([P, 4, E], f32, tag="wstage")
                if ct < 24:
                    nc.sync.dma_start(
                        w_stage[:],
                        w[col:col + SZ].rearrange("(j p) e -> p j e", p=P),
                    )
                else:
                    # rows 12288..12500: chunk 0 full, chunk 1 has 84 rows + pad
                    nc.vector.memset(w_stage[:, 1, :], 0.0)
                    nc.sync.dma_start(
                        w_stage[:, 0, :], w[col:col + P, :]
                    )
                    nc.sync.dma_start(
                        w_stage[:CPC - col - P, 1, :], w[col + P:CPC, :]
                    )

                ssq = small.tile([P, 4], f32, tag="ssq")
                for j in range(NCH):
                    sq_scr = scratch.tile([P, E], f32, tag="sq")
                    nc.vector.scalar_tensor_tensor(
                        out=sq_scr[:], in0=w_stage[:, j, :], scalar=1.0,
                        in1=w_stage[:, j, :],
                        op0=mybir.AluOpType.mult, op1=mybir.AluOpType.mult,
                        accum_out=ssq[:, j:j + 1],
                    )
                wnorm = small.tile([P, 4], f32, tag="wnorm")
                nc.scalar.activation(
                    wnorm[:, :NCH], ssq[:, :NCH],
                    mybir.ActivationFunctionType.Sqrt,
                )
                nc.vector.tensor_scalar_max(wnorm[:, :NCH], wnorm[:, :NCH], 1e-12)
                wrs = small.tile([P, 4], f32, tag="wrs")
                nc.vector.reciprocal(wrs[:, :NCH], wnorm[:, :NCH])

                wn = wn_pool.tile([P, 4, E], f32r, tag="wn")
                for j in range(NCH):
                    nc.vector.tensor_scalar(
                        out=wn[:, j, :], in0=w_stage[:, j, :],
                        scalar1=wrs[:, j:j + 1], scalar2=None,
                        op0=mybir.AluOpType.mult,
                    )

                # transpose wn -> wT [e%128, e//128, class]
                wT = wt_pool.tile([P, 4, 512], f32r, tag="wt")
                for j in range(NCH):
                    ps_t = pst_pool.tile([P, 512], f32, tag="tpose")
                    for k in range(4):
                        nc.tensor.matmul(
                            ps_t[:, k * P:(k + 1) * P].bitcast(f32r),
                            wn[:, j, k * P:(k + 1) * P],
                            ident[:],
                            is_transpose=True,
                        )
                    nc.scalar.activation(
                        wT[:, :, j * P:(j + 1) * P],
                        ps_t[:].rearrange("p (k c) -> p k c", k=4),
                        mybir.ActivationFunctionType.Copy,
                    )

                # matmuls + epilogue
                o_stage = ostage_pool.tile([P, 4, 512], f32, tag="ostage")
                for m in range(4):
                    ps_o = pso_pool.tile([P, 512], f32, tag="mout")
                    for k in range(4):
                        nc.tensor.matmul(
                            ps_o[:, :SZ],
                            embT[:, k, m * P:(m + 1) * P],
                            wT[:, k, :SZ],
                            start=(k == 0),
                            stop=(k == 3),
                        )
                    if m < 3:
                        nc.vector.tensor_copy(o_stage[:, m, :SZ], ps_o[:, :SZ])
                    else:
                        nc.scalar.activation(
                            o_stage[:, m, :SZ], ps_o[:, :SZ],
                            mybir.ActivationFunctionType.Copy,
                        )
                    exp_scr = scratch.tile([P, 512], f32, tag="exp")
                    idx = m * 32 + ct
                    nc.scalar.activation(
                        exp_scr[:, :SZ], ps_o[:, :SZ],
                        mybir.ActivationFunctionType.Exp,
                        accum_out=expsums[:, idx:idx + 1],
                    )

                nc.sync.dma_start(
                    out_v[:, :, col:col + SZ], o_stage[:, :, :SZ]
                )
                col += SZ

            # ---- final row sum-exp reduce + DMA
            se_final = persist.tile([P, 4], f32)
            for m in range(4):
                nc.vector.reduce_sum(
                    se_final[:, m:m + 1],
                    expsums[:, m * 32:m * 32 + 25],
                    axis=mybir.AxisListType.X,
                )
            nc.sync.dma_start(sumexp.rearrange("(m p) -> p m", p=P), se_final[:])

    nc.finalize()
    return nc


def _get_nc():
    if "nc" not in _CACHE:
        _CACHE["nc"] = _build()
    return _CACHE["nc"]


def kernel(embeddings, labels, weight):
    from concourse.bass_utils import run_bass_kernel_spmd

    embeddings = np.ascontiguousarray(np.asarray(embeddings, dtype=np.float32))
    weight = np.ascontiguousarray(np.asarray(weight, dtype=np.float32))
    labels_np = np.asarray(labels)

    nc = _get_nc()
    in_maps = [
        {"emb": embeddings, "w": weight[c * CPC:(c + 1) * CPC]}
        for c in range(NCORES)
    ]
    res = run_bass_kernel_spmd(nc, in_maps, core_ids=list(range(NCORES)))

    output = np.concatenate(
        [res.results[c]["out"][:, :CPC] for c in range(NCORES)], axis=1
    )  # (512, 100000) f32
    # per-core row sums of exp(out) incl. the 44 zero-pad columns (exp(0)=1)
    sumexp = np.stack([res.results[c]["sumexp"] for c in range(NCORES)])
    total = sumexp.sum(axis=0, dtype=np.float64) - NCORES * float(PAD_COLS)

    # one-hot phi fix-up on the 512 label cells
    rows = np.arange(B)
    lab = labels_np.astype(np.int64)
    cos_s = output[rows, lab].astype(np.float64)      # S * cos
    cos = cos_s / S
    sine = np.sqrt(np.clip(1.0 - cos * cos, 0.0, None))
    phi = cos * COS_M - sine * SIN_M
    phi = np.where(cos > TH, phi, cos - MM)
    target = (phi * S)
    output[rows, lab] = target.astype(np.float32)
    tgt32 = output[rows, lab].astype(np.float64)      # post-rounding value
    total = total + np.exp(tgt32) - np.exp(cos_s)

    logp = tgt32 - np.log(total)
    loss = np.float32(-logp.mean())
    return loss, output
